# revision 18
# baseline (speedup 1.0000x reference)
"""Category-specific 2-layer MLP (MoE-style routing), expert-parallel on 8 NeuronCores.

Math (per sample b with category c = cat_ids[b]):
    h   = relu(x_flat[b] @ W1[c] + b1[c])      x_flat: [32, 4096], W1: [8, 4096, 1024]
    out = h @ W2[c] + b2[c]                    W2: [8, 1024, 512]

Sharding: expert-parallel. Core k holds ONLY category k's weights and computes the
full dense MLP for all 32 samples; the host gathers row b from core cat_ids[b].

Per-core traffic is minimized with precision folding (rel-err budget is 2e-2):
  W1 -> fp8 e3m4 scaled by 128   (measured end-to-end rel err ~1.4e-2)
  x  -> bf16 scaled by 1/128     (power-of-2 scales cancel exactly in the matmul)
  W2 -> bf16
This cuts the DMA stream from ~19.4 MB fp32 to ~5.6 MB.

The stream is split over the THREE procs that can issue DMAs (SP + Activation via
HWDGE, Pool via SWDGE): in this toolchain's cost model each DMA's transfer time is
charged to the issuing engine, so three engines give three parallel streams.
NUM_HWDGE_SEMS is pinned to 2 with strictly alternating SP/Act issue order so each
engine owns one HWDGE proc (per-proc completion stays FIFO, keeping cumulative
semaphore waits sound).

Layer 1 computes hT[u] = (x @ W1)^T mid-tiles into a single PSUM tile [128, 8, 32]
(8 interleaved accumulation groups); biases (when nonzero) are folded in as an
extra K=1 matmul row (lhsT = bias segment, rhs = ones) so the PSUM evict is a
single tensor_scalar max(psum, 0) -> bf16. Layer 2 likewise, evicted by a single
tensor_copy, then stored transposed via SWDGE; the host gather undoes the
transpose for free.

The PE clock ramps to full speed only after 3us of continuous busy; any idle gap
resets it. Dummy [1, N] matmuls on a memset tile warm the PE up during the DMA
stream and bridge predicted arrival gaps so the real matmuls run at full clock.

Toolchain constraint: at most ONE sync-wait per instruction. Every W1/W2 slab has
its own SBUF tile (no slot-reuse waits); xt is issued before the W1 slabs on the
Act proc and one dummy matmul touches it (so later matmuls carry only their slab
wait); W2 halves are issued before the last W1 slabs on their procs so layer 2's
waits are covered transitively. Verified by _assert_wait_budget at build time.
"""

import numpy as np

import concourse.bass as bass
import concourse.mybir as mybir
import concourse.tile_sem_assignment as _tsa
from concourse import tile
from concourse.bass_utils import run_bass_kernel_spmd

NUM_CAT = 8
B = 32
IN_DIM = 4096   # 16 * 256
MID = 1024
OUT = 512       # 16 * 32
P = 128
KT1 = IN_DIM // P    # 32 k-tiles for layer 1
KT2 = MID // P       # 8 mid-tiles (layer-1 out / layer-2 contraction)
NT = OUT // P        # 4 out-tiles
S1 = 128.0           # power-of-2 scale: W1 *= S1 (fp8), x /= S1 (bf16)

F32 = mybir.dt.float32
BF16 = mybir.dt.bfloat16
FP8 = mybir.dt.float8e3  # e3m4

# --- per-engine slab plan -----------------------------------------------------
# Each entry is k-tiles per W1 DMA on that engine. SP additionally carries the
# first W2 half (inserted before its last two slabs), Act carries xt (first) and
# the second W2 half (before its last two slabs), Pool carries only W1 (and the
# output store + optional aux at the end). SP/Act must issue the SAME number of
# HWDGE DMAs, strictly alternating, so each owns one of the 2 HWDGE procs.
SP_SLABS = (2, 2, 2, 2, 2)    # 10 kt + W2a   -> 6 HWDGE DMAs
ACT_SLABS = (2, 2, 2, 2)      # 8 kt + xt+W2b -> 6 HWDGE DMAs
POOL_SLABS = (2, 2, 2, 2, 2, 2, 2)  # 14 kt via SWDGE
assert sum(SP_SLABS) + sum(ACT_SLABS) + sum(POOL_SLABS) == KT1
assert len(SP_SLABS) + 1 == len(ACT_SLABS) + 2

# --- cost-model constants used only for static PE schedule planning -----------
_NSPB = 0.3855421  # DMA ns per byte-per-partition (400e9/128 B/s * 0.83 util)
_DMA_MIN = 500.0
_HW_INIT = 1716.7  # HWDGE post-exec latency before data is consumable
_SW_INIT = 1883.3  # SWDGE (Pool) same
_SEM = 100.0
_T0 = 200.0        # engine streams start after the tile preamble barrier
_PE_MID = 1e9 / 1.2e9
_PE_FULL = 1e9 / 2.4e9
_RAMP_NS = 3000.0


def _dma_exec(bytes_pp: float) -> float:
    return max(bytes_pp * _NSPB, _DMA_MIN)


class _PatchHwdgeQueues:
    """Pin Tile's HWDGE round-robin to n procs during scheduling."""

    def __init__(self, n: int):
        self.n = n

    def __enter__(self):
        self._saved = _tsa.NUM_HWDGE_SEMS
        _tsa.NUM_HWDGE_SEMS = self.n
        return self

    def __exit__(self, *exc):
        _tsa.NUM_HWDGE_SEMS = self._saved
        return False


def _patch_tail_drain():
    """Split Tile's kernel-tail drain (one wait per live proc) into a chain of
    single-wait drains: this walrus build caps sync-wait commands per instruction
    and rejects the stock multi-wait drain."""
    if getattr(tile.TileContext, "_tail_drain_patched", False):
        return
    from concourse.vector_clock import ScopedClock, VectorClock

    def _drain_and_barrier(self, tick_clock, wait_clock):
        gc = tick_clock.global_clock
        n = len(gc)
        for p in range(n):
            if gc[p] <= 0:
                continue
            sub = [0] * n
            sub[p] = gc[p]
            d = self.nc.sync.drain()
            wait_clock.add_sem_waits(d.ins, ScopedClock({None: VectorClock(sub)}))
        self.nc.all_engine_barrier()
        assert self.sems is not None
        popped = self.nc._tile_sem_poison_stack.pop()
        assert popped is self._sem_poison
        self.nc.clear_and_free_semaphores(list(self.sems.allocated().values()))
        self.nc.all_engine_barrier()

    tile.TileContext._drain_and_barrier = _drain_and_barrier
    tile.TileContext._tail_drain_patched = True


_patch_tail_drain()


def _build_nc(with_bias: bool) -> bass.Bass:
    nc = bass.Bass()

    # xt[p, t, b] = x_flat[b, t*128 + p] / 128, bf16.
    xt = nc.dram_tensor("xt", [P, KT1, B], BF16, kind="ExternalInput")
    w1 = nc.dram_tensor("w1", [IN_DIM, MID], FP8, kind="ExternalInput")
    w2 = nc.dram_tensor("w2", [MID, OUT], BF16, kind="ExternalInput")
    if with_bias:
        # aux[0, :MID] = b1; aux[0, MID:MID+OUT] = b2; aux[0, MID+OUT:] = 1.0
        aux = nc.dram_tensor("aux", [1, MID + OUT + B], BF16, kind="ExternalInput")
    out = nc.dram_tensor("out", [OUT, B], F32, kind="ExternalOutput")  # transposed

    with _PatchHwdgeQueues(2), tile.TileContext(nc) as tc:
        with (
            tc.tile_pool(name="const", bufs=1) as const,
            tc.tile_pool(name="w1p", bufs=1) as w1p,
            tc.tile_pool(name="w2p", bufs=1) as w2p,
            tc.tile_pool(name="work", bufs=1) as work,
            tc.tile_pool(name="psum", bufs=1, space="PSUM") as psum,
        ):
            # Warmup data for PE dummy matmuls: DVE memsets it right away, so
            # dummies can start ~0.5us in and hold the PE clock ramp.
            warm_sb = work.tile([1, 257], BF16)
            nc.vector.memset(warm_sb[:], 1.0)

            # ---- DMA issue. HWDGE round-robin = python order; alternate SP/Act
            # strictly so SP DMAs own proc0 and Act DMAs own proc1. Pool DMAs
            # round-robin their own SWDGE procs independently.
            sp_q = []   # (tile, n_ktiles, row0)  in SP issue order
            act_q = []
            pool_q = []

            row = 0
            slabs = []  # (engine_name, tile, sz, row0) in global k-tile order

            def w1_slab(sz, name):
                nonlocal row
                t = w1p.tile([P, sz, MID], FP8, tag=name, name=name)
                r0 = row
                row += P * sz
                return (t, sz, r0)

            # Assign k-tile ranges engine by engine (order within engine =
            # issue order). Global k order: SP slabs, Act slabs, Pool slabs.
            sp_slabs = [w1_slab(sz, f"w1s{i}") for i, sz in enumerate(SP_SLABS)]
            act_slabs = [w1_slab(sz, f"w1a{i}") for i, sz in enumerate(ACT_SLABS)]
            pool_slabs = [w1_slab(sz, f"w1p{i}") for i, sz in enumerate(POOL_SLABS)]
            assert row == IN_DIM * 1  # all rows covered

            xt_sb = const.tile([P, KT1, B], BF16)
            w2a_sb = w2p.tile([P, KT2 // 2, OUT], BF16, tag="w2a", name="w2a")
            w2b_sb = w2p.tile([P, KT2 // 2, OUT], BF16, tag="w2b", name="w2b")
            if with_bias:
                aux_sb = const.tile([1, MID + OUT + B], BF16)

            # Interleaved HWDGE issue: (SP, Act) pairs.
            # SP:  s0 s1 s2 W2a s3 s4
            # Act: xt a0 a1 W2b a2 a3
            def dma_w1(eng, slab):
                t, sz, r0 = slab
                eng.dma_start(
                    t[:], w1[r0 : r0 + P * sz, :].rearrange("(f p) n -> p f n", p=P)
                )

            sp_seq = (
                [("w1", sp_slabs[0]), ("w1", sp_slabs[1]), ("w1", sp_slabs[2]),
                 ("w2a", None), ("w1", sp_slabs[3]), ("w1", sp_slabs[4])]
            )
            act_seq = (
                [("xt", None), ("w1", act_slabs[0]), ("w1", act_slabs[1]),
                 ("w2b", None), ("w1", act_slabs[2]), ("w1", act_slabs[3])]
            )
            assert len(sp_seq) == len(act_seq)

            for (sp_kind, sp_arg), (act_kind, act_arg) in zip(sp_seq, act_seq):
                # SP issue
                if sp_kind == "w1":
                    dma_w1(nc.sync, sp_arg)
                else:
                    nc.sync.dma_start(
                        w2a_sb[:],
                        w2[0 : P * (KT2 // 2), :].rearrange("(f p) n -> p f n", p=P),
                    )
                # Act issue
                if act_kind == "xt":
                    nc.scalar.dma_start(xt_sb[:], xt[:])
                elif act_kind == "w1":
                    dma_w1(nc.scalar, act_arg)
                else:
                    nc.scalar.dma_start(
                        w2b_sb[:],
                        w2[P * (KT2 // 2) :, :].rearrange("(f p) n -> p f n", p=P),
                    )

            # Pool (SWDGE) stream: all its W1 slabs, then aux (bias variant).
            for slab in pool_slabs:
                dma_w1(nc.gpsimd, slab)
            if with_bias:
                nc.gpsimd.dma_start(aux_sb[:], aux[:])

            # ---- static arrival-time plan (cost-model replica) --------------
            sp_t = _T0
            act_t = _T0
            pool_t = _T0

            vis = {}  # slab id -> (visible_time, tile, sz, kt0)
            kt0 = 0

            def kt_of(slab):
                t, sz, r0 = slab
                return r0 // P

            for kind, arg in sp_seq:
                if kind == "w1":
                    t, sz, r0 = arg
                    sp_t += _dma_exec(sz * MID * 1)
                    vis[id(arg)] = (sp_t + _HW_INIT + _SEM, t, sz, r0 // P)
                else:
                    sp_t += _dma_exec((KT2 // 2) * OUT * 2)
            xt_vis = None
            for kind, arg in act_seq:
                if kind == "w1":
                    t, sz, r0 = arg
                    act_t += _dma_exec(sz * MID * 1)
                    vis[id(arg)] = (act_t + _HW_INIT + _SEM, t, sz, r0 // P)
                elif kind == "xt":
                    act_t += _dma_exec(KT1 * B * 2)
                    xt_vis = act_t + _HW_INIT + _SEM
                else:
                    act_t += _dma_exec((KT2 // 2) * OUT * 2)
            for slab in pool_slabs:
                t, sz, r0 = slab
                pool_t += _dma_exec(sz * MID * 1)
                vis[id(slab)] = (pool_t + _SW_INIT + _SEM, t, sz, r0 // P)

            arrival = sorted(
                (vis[id(s)] for s in sp_slabs + act_slabs + pool_slabs),
                key=lambda e: e[0],
            )

            # ---- PE program ---------------------------------------------------
            # PSUM layout: concurrent matmul accumulation groups must own
            # distinct 2KB zero regions (banks): a start=True marks its whole
            # bank pending-zero, wiping sibling groups' accumulation state.
            #   tile A [P,7,512] (banks 0-6): hT[u] u=0..6 at [:,u,0:32];
            #                                  oT[v] v=0..3 at [:,v,128:160]
            #   tile B [P,1,512] (bank 7):    hT[7] at [:,0,0:32];
            #                                  dummy scratch at [0:1,0,64:320]
            # Dummies never use start=True (except the very first, which runs
            # before any real group), so they cannot poison live groups.
            psA = psum.tile([P, 7, 512], F32, tag="psA", name="psA")
            psB = psum.tile([P, 1, 512], F32, tag="psB", name="psB")

            pe_t = 520.0          # planned first-dummy start (memset + sem)
            pe_busy0 = pe_t
            SLACK = 90.0
            first_dummy = [True]

            def pe_cycle():
                return _PE_FULL if (pe_t - pe_busy0) > _RAMP_NS else _PE_MID

            # Each dummy size-class writes a fixed, disjoint byte range of
            # bank 7 so the pending-zero state stays uniform per write (the
            # hardware asserts all-or-nothing zero-region hits).
            _DUMMY_OFF = {256: 64, 64: 320, 32: 384}

            def dummy(n_free):
                nonlocal pe_t
                off = _DUMMY_OFF[n_free]
                nc.tensor.matmul(
                    psB[0:1, 0, off : off + n_free],
                    warm_sb[0:1, 256:257],
                    warm_sb[0:1, 0:n_free],
                    start=first_dummy[0],
                    stop=first_dummy[0],
                    skip_group_check=True,
                )
                first_dummy[0] = False
                pe_t += n_free * pe_cycle()

            import os
            _NO_DUMMIES = os.environ.get("K_NO_DUMMIES") == "1"

            def fill_until(target):
                # keep PE busy (no idle gap -> no clock-ramp reset) until target
                if _NO_DUMMIES:
                    return
                while pe_t < target:
                    gap = (target - pe_t) / pe_cycle()
                    n = 256 if gap > 128 else (64 if gap > 40 else 32)
                    dummy(n)

            def ht_dst(u):
                return psA[:, u, 0:B] if u < KT2 - 1 else psB[:, 0, 0:B]

            started = set()  # u groups with start consumed

            def l1_slab(tile_, sz, kt_base, is_last):
                nonlocal pe_t
                for f in range(sz):
                    for u in range(KT2):
                        # zero-bias: the very last k-row of the last slab stops
                        # each group; bias variant stops via the bias row.
                        stops = (not with_bias) and is_last and f == sz - 1
                        nc.tensor.matmul(
                            ht_dst(u),
                            tile_[:, f, P * u : P * (u + 1)],
                            xt_sb[:, kt_base + f, :],
                            start=(u not in started),
                            stop=stops,
                            skip_group_check=True,
                        )
                        started.add(u)
                        pe_t += B * pe_cycle()

            # warm up until xt is visible (xt is Act's first DMA, so it is
            # visible no later than any Act slab; SP/Pool firsts are ~equal)
            fill_until(xt_vis + SLACK)
            # touch xt so later matmuls only ever wait on their own W1 slab
            nc.tensor.matmul(
                psB[0:1, 0, 384 : 384 + B],
                warm_sb[0:1, 256:257],
                xt_sb[0:1, 0, :],
                start=False,
                stop=False,
                skip_group_check=True,
            )
            pe_t += B * pe_cycle()

            for i, (t_vis, tile_, sz, kt_base) in enumerate(arrival):
                fill_until(t_vis + SLACK)
                l1_slab(tile_, sz, kt_base, is_last=(i == len(arrival) - 1))

            if with_bias:
                for u in range(KT2):
                    nc.tensor.matmul(
                        ht_dst(u),
                        aux_sb[0:1, P * u : P * (u + 1)],
                        aux_sb[0:1, MID + OUT : MID + OUT + B],
                        start=False,
                        stop=True,
                        skip_group_check=True,
                    )
                    pe_t += B * pe_cycle()

            # Bridge dummies BEFORE the evicts are emitted: they carry no waits
            # (tile B's reader, evict_B, is emitted after them) and keep the PE
            # clock warm across the evict latency.
            fill_until(pe_t + 560.0)

            # ---- evict hT on DVE (GPSIMD cannot access PSUM): banks 0-6
            # first (layer 2 needs u=0 immediately), then bank 7. evict_B also
            # waits out the bridge dummies (WAR on tile B), which is why all
            # u=7 matmuls run at the very end of layer 2.
            ht_sbA = work.tile([P, KT2 - 1, B], BF16, tag="htA", name="htA")
            ht_sbB = work.tile([P, 1, B], BF16, tag="htB", name="htB")
            nc.vector.tensor_scalar_max(ht_sbA[:], psA[:, 0 : KT2 - 1, 0:B], 0.0)
            nc.vector.tensor_scalar_max(ht_sbB[:], psB[:, 0:1, 0:B], 0.0)

            # ---- layer 2: oT[v] at psA[:, v, 384:416] -- spare bytes of the
            # hT banks. Those bytes are still pending-zero from the hT groups'
            # start=True bank poison, so the first start=False write lands as
            # an overwrite (hardware zero-region semantics): no new group, no
            # slot reuse. Each mm's WAR vs evict_A merges with its ht_sbA data
            # wait into a single DVE wait. u=7 runs last so Pool's evict of
            # bank 7 can finish while u=0..6 stream.
            for v in range(NT):
                for u in range(KT2 - 1):
                    w2_sb = w2a_sb if u < KT2 // 2 else w2b_sb
                    nc.tensor.matmul(
                        psA[:, v, 384 : 384 + B],
                        w2_sb[:, u % (KT2 // 2), P * v : P * (v + 1)],
                        ht_sbA[:, u, :],
                        start=False,
                        stop=False,
                        skip_group_check=True,
                    )
                    pe_t += B * pe_cycle()
            for v in range(NT):
                nc.tensor.matmul(
                    psA[:, v, 384 : 384 + B],
                    w2b_sb[:, (KT2 - 1) % (KT2 // 2), P * v : P * (v + 1)],
                    ht_sbB[:, 0, :],
                    start=False,
                    stop=False,
                    skip_group_check=True,
                )
                pe_t += B * pe_cycle()
            if with_bias:
                for v in range(NT):
                    nc.tensor.matmul(
                        psA[:, v, 384 : 384 + B],
                        aux_sb[0:1, MID + P * v : MID + P * (v + 1)],
                        aux_sb[0:1, MID + OUT : MID + OUT + B],
                        start=False,
                        stop=False,
                        skip_group_check=True,
                    )
                    pe_t += B * pe_cycle()

            # ---- evict oT (fp32) and store transposed via SWDGE --------------
            ot_sb = work.tile([P, NT, B], F32, tag="otsb", name="otsb")
            nc.vector.tensor_copy(ot_sb[:], psA[:, 0:NT, 384 : 384 + B])
            nc.gpsimd.dma_start(out.rearrange("(v p) b -> p v b", p=P), ot_sb[:])

    _prune_redundant_waits(nc)
    _assert_wait_budget(nc)
    return nc


def _prune_redundant_waits(nc: bass.Bass):
    """Drop sem waits that are transitively implied by the instruction's other
    waits (Tile emits but does not transitively elide them): if instruction I
    waits on (s', v') and the instruction that fires (s', v') could only have
    dispatched after (s, v) had already fired, then I's wait on (s, v) is
    redundant. Soundness: a sem wait firing implies its producer completed,
    which implies the producer dispatched, which implies the producer's own
    waits had fired (engine streams are in-order)."""
    insts = [i for blk in nc.m.functions[0].blocks for i in blk.instructions]

    # per-sem cumulative update -> producing instruction index
    producers: dict[str, list[tuple[int, int]]] = {}  # sem -> [(cum, idx)]
    cum: dict[str, int] = {}
    engine_pred: dict[int, int] = {}
    last_on_engine: dict = {}
    for idx, inst in enumerate(insts):
        eng = inst.engine
        if eng in last_on_engine:
            engine_pred[idx] = last_on_engine[eng]
        last_on_engine[eng] = idx
        si = inst.sync_info
        if si is not None:
            for u in si.on_update:
                v = getattr(u, "update_value", 1) or 1
                cum[u.ant_name] = cum.get(u.ant_name, 0) + v
                producers.setdefault(u.ant_name, []).append(
                    (cum[u.ant_name], idx)
                )

    def resolve(sem: str, val: int) -> int | None:
        for c, idx in producers.get(sem, ()):
            if c >= val:
                return idx
        return None

    # VC[i][sem] = max count guaranteed fired when i dispatches
    VC: list[dict[str, int] | None] = [None] * len(insts)

    def vc(idx: int) -> dict[str, int]:
        if VC[idx] is not None:
            return VC[idx]
        VC[idx] = {}  # break cycles conservatively (empty = no guarantees)
        out: dict[str, int] = {}
        if idx in engine_pred:
            for s, v in vc(engine_pred[idx]).items():
                out[s] = max(out.get(s, 0), v)
        si = insts[idx].sync_info
        if si is not None:
            for w in si.on_wait:
                out[w.ant_name] = max(out.get(w.ant_name, 0), w.wait_value)
                p = resolve(w.ant_name, w.wait_value)
                if p is not None:
                    for s, v in vc(p).items():
                        out[s] = max(out.get(s, 0), v)
        VC[idx] = out
        return out

    for idx, inst in enumerate(insts):
        si = inst.sync_info
        if si is None or len(si.on_wait) <= 1:
            continue
        waits = list(si.on_wait)
        for w in waits:
            others: dict[str, int] = {}
            if idx in engine_pred:
                for s, v in vc(engine_pred[idx]).items():
                    others[s] = max(others.get(s, 0), v)
            for w2 in waits:
                if w2 is w:
                    continue
                others[w2.ant_name] = max(
                    others.get(w2.ant_name, 0), w2.wait_value
                )
                p = resolve(w2.ant_name, w2.wait_value)
                if p is not None:
                    for s, v in vc(p).items():
                        others[s] = max(others.get(s, 0), v)
            if others.get(w.ant_name, 0) >= w.wait_value:
                si.on_wait.remove(w)
                break


def _assert_wait_budget(nc: bass.Bass, max_waits: int = 1):
    """This walrus build rejects instructions with >1 sync wait; fail fast."""
    bad = []
    for blk in nc.m.functions[0].blocks:
        for inst in blk.instructions:
            if type(inst).__name__ not in (
                "InstMatmult",
                "InstDMACopy",
                "InstDrain",
                "InstTensorCopy",
                "InstTensorScalarPtr",
                "InstMemset",
            ):
                continue
            si = inst.sync_info
            nw = len(si.on_wait) if si is not None else 0
            if nw > max_waits:
                bad.append(
                    (
                        inst.name,
                        type(inst).__name__,
                        [(w.ant_name, w.wait_value) for w in si.on_wait],
                    )
                )
    if bad:
        raise RuntimeError(f"instructions with >{max_waits} sync waits: {bad}")


_NC_CACHE: dict[bool, bass.Bass] = {}


def _get_nc(with_bias: bool) -> bass.Bass:
    if with_bias not in _NC_CACHE:
        _NC_CACHE[with_bias] = _build_nc(with_bias)
    return _NC_CACHE[with_bias]


def _make_in_maps(x, W1, b1, W2, b2, with_bias: bool):
    import ml_dtypes

    x_flat = np.ascontiguousarray(np.asarray(x, dtype=np.float32)).reshape(B, IN_DIM)
    # xt[p, t, b] = x_flat[b, t*128 + p] / S1  (exact: power-of-2 scale)
    xt = np.ascontiguousarray(
        (x_flat / S1).reshape(B, KT1, P).transpose(2, 1, 0)
    ).astype(ml_dtypes.bfloat16)
    W1q = np.asarray(
        np.asarray(W1, dtype=np.float32) * S1, dtype=ml_dtypes.float8_e3m4
    )
    W2q = np.asarray(np.asarray(W2, dtype=np.float32), dtype=ml_dtypes.bfloat16)
    maps = []
    for k in range(NUM_CAT):
        m = {
            "xt": xt,
            "w1": np.ascontiguousarray(W1q[k]),
            "w2": np.ascontiguousarray(W2q[k]),
        }
        if with_bias:
            auxk = np.zeros((1, MID + OUT + B), dtype=ml_dtypes.bfloat16)
            auxk[0, :MID] = np.asarray(b1[k], dtype=np.float32)
            auxk[0, MID : MID + OUT] = np.asarray(b2[k], dtype=np.float32)
            auxk[0, MID + OUT :] = 1.0
            m["aux"] = auxk
        maps.append(m)
    return maps


def kernel(x, W1, b1, W2, b2, cat_ids) -> np.ndarray:
    with_bias = bool(
        np.any(np.asarray(b1, dtype=np.float32))
        or np.any(np.asarray(b2, dtype=np.float32))
    )
    nc = _get_nc(with_bias)
    in_maps = _make_in_maps(x, W1, b1, W2, b2, with_bias)
    res = run_bass_kernel_spmd(nc, in_maps, list(range(NUM_CAT))).results
    per_cat = np.stack(
        [np.asarray(res[k]["out"], dtype=np.float32) for k in range(NUM_CAT)]
    )  # [8, OUT, B]
    cat = np.asarray(cat_ids).astype(np.int64).reshape(B)
    sel = per_cat[cat, :, np.arange(B)]  # [B, OUT] (gather undoes the transpose)
    return np.ascontiguousarray(sel.reshape(B, 16, 32).astype(np.float32))


# revision 19
# speedup vs baseline: 1.1535x; 1.1535x over previous
"""Category-specific 2-layer MLP (MoE-style routing), expert-parallel on 8 NeuronCores.

Math (per sample b with category c = cat_ids[b]):
    h   = relu(x_flat[b] @ W1[c] + b1[c])      x_flat: [32, 4096], W1: [8, 4096, 1024]
    out = h @ W2[c] + b2[c]                    W2: [8, 1024, 512]

Sharding: expert-parallel. Core k holds ONLY category k's weights and computes the
full dense MLP for all 32 samples; the host gathers row b from core cat_ids[b].

Per-core traffic is minimized with precision folding (rel-err budget is 2e-2):
  W1 -> fp8 e3m4 scaled by 128   (measured end-to-end rel err ~1.4e-2)
  x  -> bf16 scaled by 1/128     (power-of-2 scales cancel exactly in the matmul)
  W2 -> bf16
This cuts the DMA stream from ~19.4 MB fp32 to ~5.6 MB.

The stream is split over the THREE procs that can issue DMAs (SP + Activation via
HWDGE, Pool via SWDGE): in this toolchain's cost model each DMA's transfer time is
charged to the issuing engine, so three engines give three parallel streams.
NUM_HWDGE_SEMS is pinned to 2 with strictly alternating SP/Act issue order so each
engine owns one HWDGE proc (per-proc completion stays FIFO, keeping cumulative
semaphore waits sound).

Layer 1 computes hT[u] = (x @ W1)^T mid-tiles into a single PSUM tile [128, 8, 32]
(8 interleaved accumulation groups); biases (when nonzero) are folded in as an
extra K=1 matmul row (lhsT = bias segment, rhs = ones) so the PSUM evict is a
single tensor_scalar max(psum, 0) -> bf16. Layer 2 likewise, evicted by a single
tensor_copy, then stored transposed via SWDGE; the host gather undoes the
transpose for free.

The PE clock ramps to full speed only after 3us of continuous busy; any idle gap
resets it. Dummy [1, N] matmuls on a memset tile warm the PE up during the DMA
stream and bridge predicted arrival gaps so the real matmuls run at full clock.

Toolchain constraint: at most ONE sync-wait per instruction. Every W1/W2 slab has
its own SBUF tile (no slot-reuse waits); xt is issued before the W1 slabs on the
Act proc and one dummy matmul touches it (so later matmuls carry only their slab
wait); W2 halves are issued before the last W1 slabs on their procs so layer 2's
waits are covered transitively. Verified by _assert_wait_budget at build time.
"""

import numpy as np

import concourse.bass as bass
import concourse.mybir as mybir
import concourse.tile_sem_assignment as _tsa
from concourse import tile
from concourse.bass_utils import run_bass_kernel_spmd

NUM_CAT = 8
B = 32
IN_DIM = 4096   # 16 * 256
MID = 1024
OUT = 512       # 16 * 32
P = 128
KT1 = IN_DIM // P    # 32 k-tiles for layer 1
KT2 = MID // P       # 8 mid-tiles (layer-1 out / layer-2 contraction)
NT = OUT // P        # 4 out-tiles
S1 = 128.0           # power-of-2 scale: W1 *= S1 (fp8), x /= S1 (bf16)

F32 = mybir.dt.float32
BF16 = mybir.dt.bfloat16
FP8 = mybir.dt.float8e3  # e3m4

# --- per-engine slab plan -----------------------------------------------------
# Each entry is k-tiles per W1 DMA on that engine. SP additionally carries the
# first W2 half (inserted before its last two slabs), Act carries xt (first) and
# the second W2 half (before its last two slabs), Pool carries only W1 (and the
# output store + optional aux at the end). SP/Act must issue the SAME number of
# HWDGE DMAs, strictly alternating, so each owns one of the 2 HWDGE procs.
SP_SLABS = (2, 2, 2, 2, 2)    # 10 kt + W2a   -> 6 HWDGE DMAs
ACT_SLABS = (2, 2, 2, 2)      # 8 kt + xt+W2b -> 6 HWDGE DMAs
POOL_SLABS = (2, 2, 2, 2, 2, 2, 2)  # 14 kt via SWDGE
assert sum(SP_SLABS) + sum(ACT_SLABS) + sum(POOL_SLABS) == KT1
assert len(SP_SLABS) + 1 == len(ACT_SLABS) + 2

# --- cost-model constants used only for static PE schedule planning -----------
_NSPB = 0.3855421  # DMA ns per byte-per-partition (400e9/128 B/s * 0.83 util)
_DMA_MIN = 500.0
_HW_INIT = 1716.7  # HWDGE post-exec latency before data is consumable
_SW_INIT = 1883.3  # SWDGE (Pool) same
_SEM = 100.0
_T0 = 200.0        # engine streams start after the tile preamble barrier
_PE_MID = 1e9 / 1.2e9
_PE_FULL = 1e9 / 2.4e9
_RAMP_NS = 3000.0


def _dma_exec(bytes_pp: float) -> float:
    return max(bytes_pp * _NSPB, _DMA_MIN)


class _PatchHwdgeQueues:
    """Pin Tile's HWDGE round-robin to n procs during scheduling."""

    def __init__(self, n: int):
        self.n = n

    def __enter__(self):
        self._saved = _tsa.NUM_HWDGE_SEMS
        _tsa.NUM_HWDGE_SEMS = self.n
        return self

    def __exit__(self, *exc):
        _tsa.NUM_HWDGE_SEMS = self._saved
        return False


def _patch_tail_drain():
    """Split Tile's kernel-tail drain (one wait per live proc) into a chain of
    single-wait drains: this walrus build caps sync-wait commands per instruction
    and rejects the stock multi-wait drain."""
    if getattr(tile.TileContext, "_tail_drain_patched", False):
        return
    from concourse.vector_clock import ScopedClock, VectorClock

    def _drain_and_barrier(self, tick_clock, wait_clock):
        gc = tick_clock.global_clock
        n = len(gc)
        for p in range(n):
            if gc[p] <= 0:
                continue
            sub = [0] * n
            sub[p] = gc[p]
            d = self.nc.sync.drain()
            wait_clock.add_sem_waits(d.ins, ScopedClock({None: VectorClock(sub)}))
        self.nc.all_engine_barrier()
        assert self.sems is not None
        popped = self.nc._tile_sem_poison_stack.pop()
        assert popped is self._sem_poison
        self.nc.clear_and_free_semaphores(list(self.sems.allocated().values()))
        self.nc.all_engine_barrier()

    tile.TileContext._drain_and_barrier = _drain_and_barrier
    tile.TileContext._tail_drain_patched = True


_patch_tail_drain()


def _build_nc(with_bias: bool) -> bass.Bass:
    nc = bass.Bass()

    # xt[p, t, b] = x_flat[b, t*128 + p] / 128, bf16.
    xt = nc.dram_tensor("xt", [P, KT1, B], BF16, kind="ExternalInput")
    w1 = nc.dram_tensor("w1", [IN_DIM, MID], FP8, kind="ExternalInput")
    w2 = nc.dram_tensor("w2", [MID, OUT], BF16, kind="ExternalInput")
    if with_bias:
        # aux[0, :MID] = b1; aux[0, MID:MID+OUT] = b2; aux[0, MID+OUT:] = 1.0
        aux = nc.dram_tensor("aux", [1, MID + OUT + B], BF16, kind="ExternalInput")
    out = nc.dram_tensor("out", [OUT, B], F32, kind="ExternalOutput")  # transposed

    with _PatchHwdgeQueues(2), tile.TileContext(nc) as tc:
        with (
            tc.tile_pool(name="const", bufs=1) as const,
            tc.tile_pool(name="w1p", bufs=1) as w1p,
            tc.tile_pool(name="w2p", bufs=1) as w2p,
            tc.tile_pool(name="work", bufs=1) as work,
            tc.tile_pool(name="psum", bufs=1, space="PSUM") as psum,
        ):
            # Warmup data for PE dummy matmuls: DVE memsets it right away, so
            # dummies can start ~0.5us in and hold the PE clock ramp.
            warm_sb = work.tile([1, 257], BF16)
            nc.vector.memset(warm_sb[:], 1.0)

            # ---- DMA issue. HWDGE round-robin = python order; alternate SP/Act
            # strictly so SP DMAs own proc0 and Act DMAs own proc1. Pool DMAs
            # round-robin their own SWDGE procs independently.
            sp_q = []   # (tile, n_ktiles, row0)  in SP issue order
            act_q = []
            pool_q = []

            row = 0
            slabs = []  # (engine_name, tile, sz, row0) in global k-tile order

            def w1_slab(sz, name):
                nonlocal row
                t = w1p.tile([P, sz, MID], FP8, tag=name, name=name)
                r0 = row
                row += P * sz
                return (t, sz, r0)

            # Assign k-tile ranges engine by engine (order within engine =
            # issue order). Global k order: SP slabs, Act slabs, Pool slabs.
            sp_slabs = [w1_slab(sz, f"w1s{i}") for i, sz in enumerate(SP_SLABS)]
            act_slabs = [w1_slab(sz, f"w1a{i}") for i, sz in enumerate(ACT_SLABS)]
            pool_slabs = [w1_slab(sz, f"w1p{i}") for i, sz in enumerate(POOL_SLABS)]
            assert row == IN_DIM * 1  # all rows covered

            xt_sb = const.tile([P, KT1, B], BF16)
            w2a_sb = w2p.tile([P, KT2 // 2, OUT], BF16, tag="w2a", name="w2a")
            w2b_sb = w2p.tile([P, KT2 // 2, OUT], BF16, tag="w2b", name="w2b")
            if with_bias:
                aux_sb = const.tile([1, MID + OUT + B], BF16)

            # Interleaved HWDGE issue: (SP, Act) pairs.
            # SP:  s0 s1 s2 W2a s3 s4
            # Act: xt a0 a1 W2b a2 a3
            def dma_w1(eng, slab):
                t, sz, r0 = slab
                eng.dma_start(
                    t[:], w1[r0 : r0 + P * sz, :].rearrange("(f p) n -> p f n", p=P)
                )

            sp_seq = (
                [("w1", sp_slabs[0]), ("w1", sp_slabs[1]), ("w1", sp_slabs[2]),
                 ("w2a", None), ("w1", sp_slabs[3]), ("w1", sp_slabs[4])]
            )
            act_seq = (
                [("xt", None), ("w1", act_slabs[0]), ("w1", act_slabs[1]),
                 ("w2b", None), ("w1", act_slabs[2]), ("w1", act_slabs[3])]
            )
            assert len(sp_seq) == len(act_seq)

            for (sp_kind, sp_arg), (act_kind, act_arg) in zip(sp_seq, act_seq):
                # SP issue
                if sp_kind == "w1":
                    dma_w1(nc.sync, sp_arg)
                else:
                    nc.sync.dma_start(
                        w2a_sb[:],
                        w2[0 : P * (KT2 // 2), :].rearrange("(f p) n -> p f n", p=P),
                    )
                # Act issue
                if act_kind == "xt":
                    nc.scalar.dma_start(xt_sb[:], xt[:])
                elif act_kind == "w1":
                    dma_w1(nc.scalar, act_arg)
                else:
                    nc.scalar.dma_start(
                        w2b_sb[:],
                        w2[P * (KT2 // 2) :, :].rearrange("(f p) n -> p f n", p=P),
                    )

            # Pool (SWDGE) stream: all its W1 slabs, then aux (bias variant).
            for slab in pool_slabs:
                dma_w1(nc.gpsimd, slab)
            if with_bias:
                nc.gpsimd.dma_start(aux_sb[:], aux[:])

            # ---- static arrival-time plan (cost-model replica) --------------
            sp_t = _T0
            act_t = _T0
            pool_t = _T0

            vis = {}  # slab id -> (visible_time, tile, sz, kt0)
            kt0 = 0

            def kt_of(slab):
                t, sz, r0 = slab
                return r0 // P

            for kind, arg in sp_seq:
                if kind == "w1":
                    t, sz, r0 = arg
                    sp_t += _dma_exec(sz * MID * 1)
                    vis[id(arg)] = (sp_t + _HW_INIT + _SEM, t, sz, r0 // P)
                else:
                    sp_t += _dma_exec((KT2 // 2) * OUT * 2)
            xt_vis = None
            for kind, arg in act_seq:
                if kind == "w1":
                    t, sz, r0 = arg
                    act_t += _dma_exec(sz * MID * 1)
                    vis[id(arg)] = (act_t + _HW_INIT + _SEM, t, sz, r0 // P)
                elif kind == "xt":
                    act_t += _dma_exec(KT1 * B * 2)
                    xt_vis = act_t + _HW_INIT + _SEM
                else:
                    act_t += _dma_exec((KT2 // 2) * OUT * 2)
            for slab in pool_slabs:
                t, sz, r0 = slab
                pool_t += _dma_exec(sz * MID * 1)
                vis[id(slab)] = (pool_t + _SW_INIT + _SEM, t, sz, r0 // P)

            arrival = sorted(
                (vis[id(s)] for s in sp_slabs + act_slabs + pool_slabs),
                key=lambda e: e[0],
            )

            # ---- PE program ---------------------------------------------------
            # PSUM layout: concurrent matmul accumulation groups must own
            # distinct 2KB zero regions (banks): a start=True marks its whole
            # bank pending-zero, wiping sibling groups' accumulation state.
            #   tile A [P,7,512] (banks 0-6): hT[u] u=0..6 at [:,u,0:32];
            #                                  oT[v] v=0..3 at [:,v,128:160]
            #   tile B [P,1,512] (bank 7):    hT[7] at [:,0,0:32];
            #                                  dummy scratch at [0:1,0,64:320]
            # Dummies never use start=True (except the very first, which runs
            # before any real group), so they cannot poison live groups.
            psA = psum.tile([P, 7, 512], F32, tag="psA", name="psA")
            psB = psum.tile([P, 1, 512], F32, tag="psB", name="psB")

            pe_t = 520.0          # planned first-dummy start (memset + sem)
            pe_busy0 = pe_t
            SLACK = 90.0
            first_dummy = [True]

            def pe_cycle():
                return _PE_FULL if (pe_t - pe_busy0) > _RAMP_NS else _PE_MID

            # Each dummy size-class writes a fixed, disjoint byte range of
            # bank 7 so the pending-zero state stays uniform per write (the
            # hardware asserts all-or-nothing zero-region hits).
            _DUMMY_OFF = {256: 64, 64: 320, 32: 384}

            def dummy(n_free):
                nonlocal pe_t
                off = _DUMMY_OFF[n_free]
                nc.tensor.matmul(
                    psB[0:1, 0, off : off + n_free],
                    warm_sb[0:1, 256:257],
                    warm_sb[0:1, 0:n_free],
                    start=first_dummy[0],
                    stop=first_dummy[0],
                    skip_group_check=True,
                )
                first_dummy[0] = False
                pe_t += n_free * pe_cycle()

            import os
            _NO_DUMMIES = os.environ.get("K_NO_DUMMIES") == "1"

            def fill_until(target):
                # keep PE busy (no idle gap -> no clock-ramp reset) until target
                if _NO_DUMMIES:
                    return
                while pe_t < target:
                    gap = (target - pe_t) / pe_cycle()
                    n = 256 if gap > 128 else (64 if gap > 40 else 32)
                    dummy(n)

            def ht_dst(u):
                return psA[:, u, 0:B] if u < KT2 - 1 else psB[:, 0, 0:B]

            started = set()  # u groups with start consumed

            def l1_slab(tile_, sz, kt_base, is_last):
                nonlocal pe_t
                for f in range(sz):
                    for u in range(KT2):
                        # zero-bias: the very last k-row of the last slab stops
                        # each group; bias variant stops via the bias row.
                        stops = (not with_bias) and is_last and f == sz - 1
                        nc.tensor.matmul(
                            ht_dst(u),
                            tile_[:, f, P * u : P * (u + 1)],
                            xt_sb[:, kt_base + f, :],
                            start=(u not in started),
                            stop=stops,
                            skip_group_check=True,
                        )
                        started.add(u)
                        pe_t += B * pe_cycle()

            # warm up until xt is visible (xt is Act's first DMA, so it is
            # visible no later than any Act slab; SP/Pool firsts are ~equal)
            fill_until(xt_vis + SLACK)
            # touch xt so later matmuls only ever wait on their own W1 slab
            nc.tensor.matmul(
                psB[0:1, 0, 384 : 384 + B],
                warm_sb[0:1, 256:257],
                xt_sb[0:1, 0, :],
                start=first_dummy[0],
                stop=first_dummy[0],
                skip_group_check=True,
            )
            first_dummy[0] = False
            pe_t += B * pe_cycle()

            for i, (t_vis, tile_, sz, kt_base) in enumerate(arrival):
                fill_until(t_vis + SLACK)
                l1_slab(tile_, sz, kt_base, is_last=(i == len(arrival) - 1))

            if with_bias:
                for u in range(KT2):
                    nc.tensor.matmul(
                        ht_dst(u),
                        aux_sb[0:1, P * u : P * (u + 1)],
                        aux_sb[0:1, MID + OUT : MID + OUT + B],
                        start=False,
                        stop=True,
                        skip_group_check=True,
                    )
                    pe_t += B * pe_cycle()

            # Bridge dummies BEFORE the evicts are emitted: they carry no waits
            # (tile B's reader, evict_B, is emitted after them) and keep the PE
            # clock warm across the evict latency.
            fill_until(pe_t + 560.0)

            # ---- evict hT on DVE (GPSIMD cannot access PSUM): banks 0-6
            # first (layer 2 needs u=0 immediately), then bank 7. evict_B also
            # waits out the bridge dummies (WAR on tile B), which is why all
            # u=7 matmuls run at the very end of layer 2.
            ht_sbA = work.tile([P, KT2 - 1, B], BF16, tag="htA", name="htA")
            ht_sbB = work.tile([P, 1, B], BF16, tag="htB", name="htB")
            nc.vector.tensor_scalar_max(ht_sbA[:], psA[:, 0 : KT2 - 1, 0:B], 0.0)
            nc.vector.tensor_scalar_max(ht_sbB[:], psB[:, 0:1, 0:B], 0.0)

            # ---- layer 2: oT[v] at psA[:, v, 384:416] -- spare bytes of the
            # hT banks. Those bytes are still pending-zero from the hT groups'
            # start=True bank poison, so the first start=False write lands as
            # an overwrite (hardware zero-region semantics): no new group, no
            # slot reuse. Each mm's WAR vs evict_A merges with its ht_sbA data
            # wait into a single DVE wait. u=7 runs last so Pool's evict of
            # bank 7 can finish while u=0..6 stream.
            for v in range(NT):
                for u in range(KT2 - 1):
                    w2_sb = w2a_sb if u < KT2 // 2 else w2b_sb
                    nc.tensor.matmul(
                        psA[:, v, 384 : 384 + B],
                        w2_sb[:, u % (KT2 // 2), P * v : P * (v + 1)],
                        ht_sbA[:, u, :],
                        start=False,
                        stop=False,
                        skip_group_check=True,
                    )
                    pe_t += B * pe_cycle()
            for v in range(NT):
                nc.tensor.matmul(
                    psA[:, v, 384 : 384 + B],
                    w2b_sb[:, (KT2 - 1) % (KT2 // 2), P * v : P * (v + 1)],
                    ht_sbB[:, 0, :],
                    start=False,
                    stop=False,
                    skip_group_check=True,
                )
                pe_t += B * pe_cycle()
            if with_bias:
                for v in range(NT):
                    nc.tensor.matmul(
                        psA[:, v, 384 : 384 + B],
                        aux_sb[0:1, MID + P * v : MID + P * (v + 1)],
                        aux_sb[0:1, MID + OUT : MID + OUT + B],
                        start=False,
                        stop=False,
                        skip_group_check=True,
                    )
                    pe_t += B * pe_cycle()

            # ---- evict oT (fp32) and store transposed via SWDGE --------------
            ot_sb = work.tile([P, NT, B], F32, tag="otsb", name="otsb")
            nc.vector.tensor_copy(ot_sb[:], psA[:, 0:NT, 384 : 384 + B])
            nc.gpsimd.dma_start(out.rearrange("(v p) b -> p v b", p=P), ot_sb[:])

    _prune_redundant_waits(nc)
    _assert_wait_budget(nc)
    return nc


def _prune_redundant_waits(nc: bass.Bass):
    """Drop sem waits that are transitively implied by the instruction's other
    waits (Tile emits but does not transitively elide them): if instruction I
    waits on (s', v') and the instruction that fires (s', v') could only have
    dispatched after (s, v) had already fired, then I's wait on (s, v) is
    redundant. Soundness: a sem wait firing implies its producer completed,
    which implies the producer dispatched, which implies the producer's own
    waits had fired (engine streams are in-order)."""
    insts = [i for blk in nc.m.functions[0].blocks for i in blk.instructions]

    # per-sem cumulative update -> producing instruction index
    producers: dict[str, list[tuple[int, int]]] = {}  # sem -> [(cum, idx)]
    cum: dict[str, int] = {}
    engine_pred: dict[int, int] = {}
    last_on_engine: dict = {}
    for idx, inst in enumerate(insts):
        eng = inst.engine
        if eng in last_on_engine:
            engine_pred[idx] = last_on_engine[eng]
        last_on_engine[eng] = idx
        si = inst.sync_info
        if si is not None:
            for u in si.on_update:
                v = getattr(u, "update_value", 1) or 1
                cum[u.ant_name] = cum.get(u.ant_name, 0) + v
                producers.setdefault(u.ant_name, []).append(
                    (cum[u.ant_name], idx)
                )

    def resolve(sem: str, val: int) -> int | None:
        for c, idx in producers.get(sem, ()):
            if c >= val:
                return idx
        return None

    # VC[i][sem] = max count guaranteed fired when i dispatches
    VC: list[dict[str, int] | None] = [None] * len(insts)

    def vc(idx: int) -> dict[str, int]:
        if VC[idx] is not None:
            return VC[idx]
        VC[idx] = {}  # break cycles conservatively (empty = no guarantees)
        out: dict[str, int] = {}
        if idx in engine_pred:
            for s, v in vc(engine_pred[idx]).items():
                out[s] = max(out.get(s, 0), v)
        si = insts[idx].sync_info
        if si is not None:
            for w in si.on_wait:
                out[w.ant_name] = max(out.get(w.ant_name, 0), w.wait_value)
                p = resolve(w.ant_name, w.wait_value)
                if p is not None:
                    for s, v in vc(p).items():
                        out[s] = max(out.get(s, 0), v)
        VC[idx] = out
        return out

    for idx, inst in enumerate(insts):
        si = inst.sync_info
        if si is None or len(si.on_wait) <= 1:
            continue
        waits = list(si.on_wait)
        for w in waits:
            others: dict[str, int] = {}
            if idx in engine_pred:
                for s, v in vc(engine_pred[idx]).items():
                    others[s] = max(others.get(s, 0), v)
            for w2 in waits:
                if w2 is w:
                    continue
                others[w2.ant_name] = max(
                    others.get(w2.ant_name, 0), w2.wait_value
                )
                p = resolve(w2.ant_name, w2.wait_value)
                if p is not None:
                    for s, v in vc(p).items():
                        others[s] = max(others.get(s, 0), v)
            if others.get(w.ant_name, 0) >= w.wait_value:
                si.on_wait.remove(w)
                break


def _assert_wait_budget(nc: bass.Bass, max_waits: int = 1):
    """This walrus build rejects instructions with >1 sync wait; fail fast."""
    bad = []
    for blk in nc.m.functions[0].blocks:
        for inst in blk.instructions:
            if type(inst).__name__ not in (
                "InstMatmult",
                "InstDMACopy",
                "InstDrain",
                "InstTensorCopy",
                "InstTensorScalarPtr",
                "InstMemset",
            ):
                continue
            si = inst.sync_info
            nw = len(si.on_wait) if si is not None else 0
            if nw > max_waits:
                bad.append(
                    (
                        inst.name,
                        type(inst).__name__,
                        [(w.ant_name, w.wait_value) for w in si.on_wait],
                    )
                )
    if bad:
        raise RuntimeError(f"instructions with >{max_waits} sync waits: {bad}")


_NC_CACHE: dict[bool, bass.Bass] = {}


def _get_nc(with_bias: bool) -> bass.Bass:
    if with_bias not in _NC_CACHE:
        _NC_CACHE[with_bias] = _build_nc(with_bias)
    return _NC_CACHE[with_bias]


def _make_in_maps(x, W1, b1, W2, b2, with_bias: bool):
    import ml_dtypes

    x_flat = np.ascontiguousarray(np.asarray(x, dtype=np.float32)).reshape(B, IN_DIM)
    # xt[p, t, b] = x_flat[b, t*128 + p] / S1  (exact: power-of-2 scale)
    xt = np.ascontiguousarray(
        (x_flat / S1).reshape(B, KT1, P).transpose(2, 1, 0)
    ).astype(ml_dtypes.bfloat16)
    W1q = np.asarray(
        np.asarray(W1, dtype=np.float32) * S1, dtype=ml_dtypes.float8_e3m4
    )
    W2q = np.asarray(np.asarray(W2, dtype=np.float32), dtype=ml_dtypes.bfloat16)
    maps = []
    for k in range(NUM_CAT):
        m = {
            "xt": xt,
            "w1": np.ascontiguousarray(W1q[k]),
            "w2": np.ascontiguousarray(W2q[k]),
        }
        if with_bias:
            auxk = np.zeros((1, MID + OUT + B), dtype=ml_dtypes.bfloat16)
            auxk[0, :MID] = np.asarray(b1[k], dtype=np.float32)
            auxk[0, MID : MID + OUT] = np.asarray(b2[k], dtype=np.float32)
            auxk[0, MID + OUT :] = 1.0
            m["aux"] = auxk
        maps.append(m)
    return maps


def kernel(x, W1, b1, W2, b2, cat_ids) -> np.ndarray:
    with_bias = bool(
        np.any(np.asarray(b1, dtype=np.float32))
        or np.any(np.asarray(b2, dtype=np.float32))
    )
    nc = _get_nc(with_bias)
    in_maps = _make_in_maps(x, W1, b1, W2, b2, with_bias)
    res = run_bass_kernel_spmd(nc, in_maps, list(range(NUM_CAT))).results
    per_cat = np.stack(
        [np.asarray(res[k]["out"], dtype=np.float32) for k in range(NUM_CAT)]
    )  # [8, OUT, B]
    cat = np.asarray(cat_ids).astype(np.int64).reshape(B)
    sel = per_cat[cat, :, np.arange(B)]  # [B, OUT] (gather undoes the transpose)
    return np.ascontiguousarray(sel.reshape(B, 16, 32).astype(np.float32))
